# revision 27
# baseline (speedup 1.0000x reference)
"""DeformMCALayer Trainium2 kernel: 8-way data-parallel over batch.

kernel(**inputs) takes the FULL inputs (x [8,256,64,64], offset_w [18,256,3,3],
deform_w [256,256,3,3]) and returns the FULL output [8,256,64,64] (float32).

Per core (one image):
  1. x cast to bf16 (SWDGE cast DMA) -> xbf; offset conv (3x3 bf16 matmuls,
     PSUM-accumulated), offsets PE-transposed per conv chunk
  2. DVE index/bilinear-weight math (floor via 1.5*2^23 magic, clip, validity
     masks). Out-of-bounds corners are weight-redistributed (ex/ey edge
     tricks) so ONE clipped flat index per (tap,pixel) suffices.
  3. xbf transposed to pixel-major via 32 xbar DMA-transposes, then
     duplicated 4x into xt4 DRAM rows: row r holds pixels [r,r+1,r+64,r+65]
     (the 4 bilinear corners) so one SWDGE gather descriptor fetches all
     4 corners (2KB).
  4. w4 bilinear weights DMA'd to DRAM pixel-major, reloaded as w4t2
     [18,2,4096] rows.
  5. main loop per (qt,k): TWO 512-idx TRANSPOSING dma_gathers (xbar spray)
     land the 4 corners channel-on-partition/pixel-on-free; PE one-hot
     sel-matmuls broadcast the per-pixel weight rows to 128 partitions
     (ACT copies PSUM->SBUF bf16); ONE DVE broadcast tensor_tensor mult
     (stride-0 cb dim) weights all 4 corners; 2 DVE adds fold corner pairs
     into patchA/patchB. No PE transposes, no per-corner tensor_scalars.
  6. deform matmul: 36-step PSUM accumulation (k x cb x A/B) vs w2;
     channel attention (mean + unbiased std -> sigmoid) fused on ACT
     reading PSUM; final scale + store
"""
import sys
sys.path.insert(0, "/opt/trn_rl_repo")
import numpy as np
import ml_dtypes

import concourse.bacc as bacc
import concourse.mybir as mybir
from concourse.tile import TileContext
from concourse.ap import AP

F32 = mybir.dt.float32
BF16 = mybir.dt.bfloat16
I16 = mybir.dt.int16
OP = mybir.AluOpType
AF = mybir.ActivationFunctionType

H = W = 64
HW = 4096
K = 9
NQT = 4
QPIX = 1024
XT4ROWS = 4161   # physical rows 0..4160; gather reads rows 65..4160
N_CORES = 8
MAGIC = float(3 * 2 ** 22)  # 1.5*2^23 round-to-int magic (|x| < 2^22)


def _mk(ap_or_handle, extra_offset, dims):
    if isinstance(ap_or_handle, AP):
        t, off = ap_or_handle.tensor, ap_or_handle.offset
    else:
        a = ap_or_handle.ap()
        t, off = a.tensor, a.offset
    return AP(t, off + extra_offset, [list(d) for d in dims])


def build_program(repeat=1):
    nc = bacc.Bacc("TRN2", target_bir_lowering=False, debug=False, num_devices=1,
                   num_swdge_queues=4)

    x_d = nc.dram_tensor("x", [256, HW], F32, kind="ExternalInput")
    offw_d = nc.dram_tensor("offw", [128, 2, K, 18], BF16, kind="ExternalInput")
    w2_d = nc.dram_tensor("w2", [128, 18, 256], BF16, kind="ExternalInput")
    basey_d = nc.dram_tensor("basey", [128, 32, K], F32, kind="ExternalInput")
    basex_d = nc.dram_tensor("basex", [128, 32, K], F32, kind="ExternalInput")
    idf_d = nc.dram_tensor("idf", [128, 128], F32, kind="ExternalInput")
    idb_d = nc.dram_tensor("idb", [128, 128], BF16, kind="ExternalInput")
    sel_d = nc.dram_tensor("sel", [36, 36, 128], BF16, kind="ExternalInput")

    xbf_d = nc.dram_tensor("xbf", [256, HW], BF16, kind="Internal")
    xt4_d = nc.dram_tensor("xt4", [XT4ROWS, 1024], BF16, kind="Internal")
    y_d = nc.dram_tensor("y", [256, HW], BF16, kind="ExternalOutput")

    with TileContext(nc) as tc:
        for _rep in range(repeat):
            with tc.tile_pool(name="const", bufs=1) as cpool:
                w2_sb = cpool.tile([128, 18, 256], BF16)
                nc.sync.dma_start(w2_sb[:], w2_d[:])
                sel_sb = cpool.tile([36, 36, 128], BF16)
                nc.sync.dma_start(sel_sb[:], sel_d[:])
                idxw = cpool.tile([128, K, NQT, 64], I16)
                # w4T rows: w4T_sb[k*4+s, blk, p] = bilinear weight of pixel
                # blk*128+p for (tap k, corner s); (blk,p) = contiguous pix
                w4T_sb = cpool.tile([36, 32, 128], BF16)

                # idx pool opened BEFORE boot pool: its tiles sit below the
                # boot tiles in SBUF, so the index math never waits on an
                # anti-dependency against released boot buffers.
                with tc.tile_pool(name="idx", bufs=1) as ipool, \
                     tc.tile_pool(name="boot", bufs=1) as bpool, \
                     tc.tile_pool(name="psconv", bufs=2, space="PSUM") as psconv, \
                     tc.tile_pool(name="pstp", bufs=2, space="PSUM") as pstp:
                    idb_sb = ipool.tile([128, 128], BF16, name="idb")
                    nc.sync.dma_start(idb_sb[:], idb_d[:])
                    offw_sb = ipool.tile([128, 2, K, 18], BF16, name="offw")
                    nc.sync.dma_start(offw_sb[:], offw_d[:])
                    idf_sb = ipool.tile([128, 128], F32, name="idf")
                    nc.sync.dma_start(idf_sb[:], idf_d[:])
                    idxw16 = ipool.tile([16, K, NQT, 64], I16, name="idxw16")

                    # zero the xt4 tail corners no data write covers (0-weight
                    # corners must read 0.0, never NaN garbage): col1: row
                    # 4160; col2: rows 4097..4160; col3: rows 4096..4160
                    zpad = ipool.tile([65, 256], BF16, name="zpad")
                    nc.vector.memset(zpad[:], 0.0)
                    nc.sync.dma_start(
                        _mk(xt4_d, 4160 * 1024 + 256, [[1024, 1], [1, 256]]),
                        zpad[0:1, :])
                    nc.sync.dma_start(
                        _mk(xt4_d, 4097 * 1024 + 512, [[1024, 64], [1, 256]]),
                        zpad[0:64, :])
                    nc.sync.dma_start(
                        _mk(xt4_d, 4096 * 1024 + 768, [[1024, 65], [1, 256]]),
                        zpad[:])

                    basey_sb = ipool.tile([128, 32, K], F32, name="basey")
                    nc.sync.dma_start(basey_sb[:], basey_d[:])
                    basex_sb = ipool.tile([128, 32, K], F32, name="basex")
                    nc.sync.dma_start(basex_sb[:], basex_d[:])
                    offT = ipool.tile([128, 32, 18], F32, name="offT")
                    w4 = ipool.tile([128, 32, K, 4], BF16, name="w4")

                    # warm the PE HAM clock gate during the input DMAs so the
                    # conv runs at 2.4 GHz
                    for _w in range(24):
                        ps_w = pstp.tile([128, 128], BF16, tag="tpw4")
                        nc.tensor.transpose(ps_w[:], idb_sb[:], idb_sb[:])

                    # conv input (padded borders): load x f32 on the sync
                    # HWDGE queue, cast to bf16 on DVE (keeps SWDGE free so
                    # the 72 gathers own all 8 DMASW sem slots with a fixed
                    # queue-per-slot map)
                    x_pad = bpool.tile([128, 2, 66, 66], BF16)
                    nc.vector.memset(x_pad[:, :, 0, :], 0.0)
                    nc.vector.memset(x_pad[:, :, 65, :], 0.0)
                    nc.vector.memset(x_pad[:, :, 1:65, 0:1], 0.0)
                    nc.vector.memset(x_pad[:, :, 1:65, 65:66], 0.0)
                    x_sb = bpool.tile([128, 2, HW], F32)
                    for cb in range(2):
                        nc.sync.dma_start(
                            x_sb[:, cb, :],
                            _mk(x_d, cb * 128 * HW, [[HW, 128], [1, HW]]))
                    xbf_sb = bpool.tile([128, 2, HW], BF16)
                    nc.vector.tensor_copy(xbf_sb[:], x_sb[:])
                    xpi = _mk(x_pad[:], 67,
                              [list(x_pad[:].ap[0]), [4356, 2], [66, 64],
                               [1, 64]])
                    xbi = _mk(xbf_sb[:], 0,
                              [list(xbf_sb[:].ap[0]), [HW, 2], [64, 64],
                               [1, 64]])
                    nc.vector.tensor_copy(xpi, xbi)
                    nc.scalar.dma_start(
                        _mk(xbf_d, 0, [[HW, 128], [128 * HW, 2], [1, HW]]),
                        xbf_sb[:])

                    # pixel-major bf16 x via 32 xbar DMA-transposes (sync
                    # HWDGE queue, overlapping the conv on PE)
                    xT_sb = bpool.tile([128, 32, 256], BF16)
                    for b in range(32):
                        src = _mk(xbf_d, b * 128, [[HW, 256], [1, 128]])
                        nc.sync.dma_start_transpose(xT_sb[:, b, :], src)
                    # duplicate 4x into xt4: pixel j=128b+p lands at
                    # col c, physical row j + {65,64,1,0}[c]; writes split
                    # across the gpsimd + scalar DMA queues to overlap
                    for c, pr0 in enumerate([65, 64, 1, 0]):
                        dst = _mk(xt4_d, pr0 * 1024 + c * 256,
                                  [[1024, 128], [128 * 1024, 32], [1, 256]])
                        eng = nc.sync if c < 2 else nc.scalar
                        eng.dma_start(dst, xT_sb[:])

                    off_sb = bpool.tile([18, HW], F32)
                    for chk in range(8):
                        ps_conv = psconv.tile([18, 512], F32, tag="conv")
                        r0 = chk * 8
                        idx = 0
                        for cb in range(2):
                            for k in range(K):
                                ky, kx = k // 3, k % 3
                                rhs = x_pad[:, cb, r0 + ky: r0 + ky + 8, kx: kx + 64]
                                nc.tensor.matmul(
                                    ps_conv[:], offw_sb[:, cb, k, :], rhs,
                                    start=(idx == 0), stop=(idx == 17))
                                idx += 1
                        nc.scalar.copy(off_sb[:, chk * 512:(chk + 1) * 512], ps_conv[:])
                        for bb in range(4):
                            b = chk * 4 + bb
                            ps_t = pstp.tile([128, 18], F32, tag="tp18")
                            nc.tensor.transpose(
                                ps_t[:], off_sb[:, b * 128:(b + 1) * 128],
                                idf_sb[0:18, 0:18])
                            nc.scalar.copy(offT[:, b, :], ps_t[:])

                    # ---------------- index & weight math ----------------
                    def it(name):
                        return ipool.tile([128, 32, K], F32, tag=name, name=name)

                    sy = it("sy"); sx = it("sx")
                    nc.vector.tensor_add(sy[:], basey_sb[:], offT[:, :, 0:K])
                    nc.vector.tensor_add(sx[:], basex_sb[:], offT[:, :, K:18])

                    def floor_(s_t, name):
                        t = it(name + "_t"); c = it(name + "_c")
                        f = it(name + "_f"); l = it(name + "_l")
                        nc.vector.tensor_scalar_add(t[:], s_t[:], MAGIC)
                        nc.vector.tensor_scalar_sub(t[:], t[:], MAGIC)
                        nc.vector.tensor_tensor(c[:], t[:], s_t[:], OP.is_gt)
                        nc.vector.tensor_sub(f[:], t[:], c[:])
                        nc.vector.tensor_sub(l[:], s_t[:], f[:])
                        return f, l

                    y0, ly = floor_(sy, "y")
                    x0, lx = floor_(sx, "x")

                    yc0 = it("yc0"); xc0 = it("xc0")
                    nc.vector.tensor_scalar(yc0[:], y0[:], 0.0, 63.0, OP.max, OP.min)
                    nc.vector.tensor_scalar(xc0[:], x0[:], 0.0, 63.0, OP.max, OP.min)

                    # masks: v = in [0,63], c62 = in [0,62], e = (== -1)
                    def masks(src_t, name):
                        a = it(name + "_a"); b = it(name + "_b"); v = it(name + "_v")
                        c2 = it(name + "_c2"); c62 = it(name + "_c62")
                        e = it(name + "_e")
                        nc.vector.tensor_scalar(a[:], src_t[:], 0.0, None, OP.is_ge)
                        nc.vector.tensor_scalar(b[:], src_t[:], 63.0, None, OP.is_le)
                        nc.vector.tensor_mul(v[:], a[:], b[:])
                        nc.vector.tensor_scalar(c2[:], src_t[:], 62.0, None, OP.is_le)
                        nc.vector.tensor_mul(c62[:], a[:], c2[:])
                        nc.vector.tensor_scalar(e[:], src_t[:], -1.0, None, OP.is_equal)
                        return v, c62, e

                    vy0, cy62, ey = masks(y0, "my")
                    vx0, cx62, ex = masks(x0, "mx")

                    oly = it("oly"); olx = it("olx")
                    nc.vector.tensor_scalar(oly[:], ly[:], -1.0, 1.0, OP.mult, OP.add)
                    nc.vector.tensor_scalar(olx[:], lx[:], -1.0, 1.0, OP.mult, OP.add)

                    # wy0 = (1-ly)*vy0 + ly*ey ; wy1 = ly*cy62 (same for x)
                    wy0 = it("wy0"); wy1 = it("wy1"); wx0 = it("wx0"); wx1 = it("wx1")
                    t1 = it("t1"); t2 = it("t2")
                    nc.vector.tensor_mul(t1[:], oly[:], vy0[:])
                    nc.vector.tensor_mul(t2[:], ly[:], ey[:])
                    nc.vector.tensor_add(wy0[:], t1[:], t2[:])
                    nc.vector.tensor_mul(wy1[:], ly[:], cy62[:])
                    nc.vector.tensor_mul(t1[:], olx[:], vx0[:])
                    nc.vector.tensor_mul(t2[:], lx[:], ex[:])
                    nc.vector.tensor_add(wx0[:], t1[:], t2[:])
                    nc.vector.tensor_mul(wx1[:], lx[:], cx62[:])

                    for s, (a_t, b_t) in enumerate([(wy0, wx0), (wy0, wx1),
                                                    (wy1, wx0), (wy1, wx1)]):
                        nc.vector.tensor_tensor(w4[:, :, :, s], a_t[:], b_t[:], OP.mult)

                    # w4 [128pix, blk, (k,s)] -> w4T [36(k,s), blk, 128]
                    # via 32 PE transposes (pix-major weight rows on-chip)
                    for blk in range(32):
                        pswt = pstp.tile([128, 128], BF16, tag="tpw4")
                        nc.tensor.transpose(
                            pswt[0:36, :],
                            _mk(w4[:], blk * 36,
                                [list(w4[:].ap[0]), [1, 36]]),
                            idb_sb[:])
                        nc.scalar.copy(w4T_sb[:, blk, :], pswt[0:36, :])

                    # physical row index: yc0*64 + xc0 + 65
                    ida = it("ida"); m1 = it("m1")
                    nc.vector.tensor_scalar(m1[:], yc0[:], 64.0, 65.0, OP.mult, OP.add)
                    nc.vector.tensor_add(ida[:], m1[:], xc0[:])

                    # reorder [p, blk=qt*8+g, k] -> idxf [p, qt, k, g]
                    idxf = ipool.tile([128, NQT, K, 8], F32, tag="idxf")
                    src_ap = _mk(ida[:], 0, [list(ida[:].ap[0]), [8 * K, NQT],
                                             [1, K], [K, 8]])
                    dst_ap = _mk(idxf[:], 0, [list(idxf[:].ap[0]), [72, NQT],
                                              [8, K], [1, 8]])
                    nc.vector.tensor_copy(dst_ap, src_ap)

                    # two-stage 16-wide transpose into SWDGE index layout,
                    # one 72-wide chunk per qt
                    T1_sb = ipool.tile([128, NQT, 128], F32, tag="T1")
                    for ch in range(NQT):
                        ps = pstp.tile([128, 128], F32, tag="tpw")
                        in_ap = _mk(idxf[:], ch * 72, [list(idxf[:].ap[0]), [1, 72]])
                        nc.tensor.transpose(ps[0:72, :], in_ap, idf_sb[:])
                        nc.scalar.copy(T1_sb[0:72, ch, :], ps[0:72, :])
                    for ch in range(NQT):
                        for q in range(8):
                            ps2f = pstp.tile([128, 128], F32, tag="tpw")
                            in2 = T1_sb[:, ch, q * 16: q * 16 + 16]
                            nc.tensor.transpose(ps2f[0:16, :], in2, idf_sb[:])
                            base = idxw16[:].offset + ch * 64 + q
                            pa = list(idxw16[:].ap[0])
                            pa[1] = 16
                            dims = [pa, [256, K], [8, 8]]
                            dst_ap = AP(idxw16[:].tensor, base, dims)
                            nc.vector.tensor_copy(dst_ap, ps2f[0:16, 0:72])
                    for cgrp in range(8):
                        nc.sync.dma_start(idxw[cgrp * 16:(cgrp + 1) * 16], idxw16[:])

                # ---------------- main: gather, weight, matmul ----------
                with tc.tile_pool(name="main", bufs=2) as mpool, \
                     tc.tile_pool(name="ybuf", bufs=1) as ypool, \
                     tc.tile_pool(name="pswb", bufs=4, space="PSUM") as pswb, \
                     tc.tile_pool(name="psmm", bufs=4, space="PSUM") as psmm:
                    y_sb = ypool.tile([128, 2, HW], BF16)
                    s1p = ypool.tile([128, 2, 8], F32, name="s1p")
                    s2p = ypool.tile([128, 2, 8], F32, name="s2p")
                    gsrc_ap = _mk(xt4_d, 0, [[1024, XT4ROWS], [1, 1024]])

                    it_no = 0
                    for qt in range(NQT):
                        # patch [128, 9k, 2h, 2cb, 512] bf16 (corner sum)
                        patchA = mpool.tile([128, K, 2, 2, 512], BF16,
                                            tag="patchA", bufs=1)

                        for k in range(K):
                            # TWO 512-idx transposing gathers: gxT layout
                            # [128c, 2h, 8e, 512i], e = s*2+cb
                            # per-queue tags: a tag's SWDGE completion
                            # semaphore (DMASW<q>) is locked to one queue, so
                            # each queue gets its own gxT tag (4-deep rotate)
                            # transposed gathers are only reliable on ONE
                            # SWDGE queue (multi-queue runs were flaky on HW:
                            # DMASW sem slots are shared across queues)
                            qn = 0
                            gxT = mpool.tile([128, 2, 8, 512], BF16,
                                             tag="gxT", bufs=4)
                            for h in range(2):
                                nc.gpsimd.dma_gather(
                                    gxT[:, h, :, :], gsrc_ap,
                                    idxw[:, k, qt, 32 * h:32 * h + 32],
                                    512, 512, 1024, elem_step=1024,
                                    transpose=True, queue_num=qn)
                            it_no += 1

                            # weight rows for (qt, k): PE sel-matmul
                            # broadcast w4T[k*4+s] row (contract over all
                            # 36 partitions, one-hot lhsT) -> 8 chunks of
                            # [128, 512]; ACT copies PSUM -> wbc bf16
                            wbc = mpool.tile([128, 4, 1024], BF16,
                                             tag="wbc", bufs=2)
                            for s4 in range(4):
                                for jc in range(2):
                                    psb = pswb.tile([128, 512], F32,
                                                    tag="wb")
                                    rhs = _mk(w4T_sb[:],
                                              (qt * 8 + jc * 4) * 128,
                                              [list(w4T_sb[:].ap[0]),
                                               [1, 512]])
                                    nc.tensor.matmul(
                                        psb[:],
                                        sel_sb[:, k * 4 + s4, :], rhs,
                                        start=True, stop=True)
                                    nc.scalar.copy(
                                        wbc[:, s4,
                                            jc * 512:(jc + 1) * 512],
                                        psb[:])

                            # broadcast mult: gxT *= wbc (in place), one op
                            # per gather half; in1 cb dim stride 0 (ISA caps
                            # APs at 3 free dims)
                            for h in range(2):
                                in0 = _mk(gxT[:], h * 4096,
                                          [list(gxT[:].ap[0]), [1, 4096]])
                                in1 = _mk(wbc[:], h * 512,
                                          [list(wbc[:].ap[0]),
                                           [1024, 4], [0, 2], [1, 512]])
                                nc.vector.tensor_tensor(in0, in0, in1, OP.mult)

                            # corner adds on DVE: s0+=s1, s2+=s3 (in
                            # place), then patch_k = s0 + s2 -> 18-step mm
                            pa = _mk(patchA[:], k * 2048,
                                     [list(patchA[:].ap[0]), [1, 2048]])
                            s0 = _mk(gxT[:], 0 * 1024,
                                     [list(gxT[:].ap[0]), [4096, 2], [1, 1024]])
                            s1 = _mk(gxT[:], 1 * 1024,
                                     [list(gxT[:].ap[0]), [4096, 2], [1, 1024]])
                            s2 = _mk(gxT[:], 2 * 1024,
                                     [list(gxT[:].ap[0]), [4096, 2], [1, 1024]])
                            s3 = _mk(gxT[:], 3 * 1024,
                                     [list(gxT[:].ap[0]), [4096, 2], [1, 1024]])
                            nc.vector.tensor_tensor(s0, s0, s1, OP.add)
                            nc.vector.tensor_tensor(s2, s2, s3, OP.add)
                            nc.vector.tensor_tensor(pa, s0, s2, OP.add)

                        # deform matmul: 18 steps (k x cb) per
                        # (h-chunk, oh); psum [128, 512]
                        for h in range(2):
                            for oh in range(2):
                                psd = psmm.tile([128, 512], F32, tag="mm")
                                idx = 0
                                for k in range(K):
                                    for cb in range(2):
                                        lhs = w2_sb[:, k * 2 + cb,
                                                    oh * 128:(oh + 1) * 128]
                                        rhs = _mk(patchA[:],
                                                  k * 2048 + h * 1024
                                                  + cb * 512,
                                                  [list(patchA[:].ap[0]),
                                                   [1, 512]])
                                        nc.tensor.matmul(
                                            psd[:], lhs, rhs,
                                            start=(idx == 0),
                                            stop=(idx == 17))
                                        idx += 1
                                cidx = qt * 2 + h
                                nc.scalar.activation(
                                    y_sb[:, oh, qt * 1024 + h * 512:
                                         qt * 1024 + (h + 1) * 512],
                                    psd[:], AF.Copy,
                                    accum_out=s1p[:, oh, cidx:cidx + 1])
                                sqscr = mpool.tile([128, 512], BF16, tag="sq")
                                nc.scalar.activation(
                                    sqscr[:], psd[:], AF.Square,
                                    accum_out=s2p[:, oh, cidx:cidx + 1])

                    # ---------------- stats + scale ----------------
                    s1 = ypool.tile([128, 2], F32)
                    s2 = ypool.tile([128, 2], F32)
                    nc.vector.reduce_sum(s1[:], s1p[:], axis=mybir.AxisListType.X)
                    nc.vector.reduce_sum(s2[:], s2p[:], axis=mybir.AxisListType.X)
                    mean = ypool.tile([128, 2], F32)
                    nc.vector.tensor_scalar_mul(mean[:], s1[:], 1.0 / HW)
                    ss = ypool.tile([128, 2], F32)
                    nc.vector.tensor_mul(ss[:], s1[:], s1[:])
                    va = ypool.tile([128, 2], F32)
                    vb = ypool.tile([128, 2], F32)
                    var = ypool.tile([128, 2], F32)
                    nc.vector.tensor_scalar_mul(va[:], s2[:], 1.0 / (HW - 1))
                    nc.vector.tensor_scalar_mul(vb[:], ss[:], 1.0 / (HW * (HW - 1.0)))
                    nc.vector.tensor_sub(var[:], va[:], vb[:])
                    nc.vector.tensor_scalar_max(var[:], var[:], 0.0)
                    std = ypool.tile([128, 2], F32)
                    nc.scalar.sqrt(std[:], var[:])
                    arg = ypool.tile([128, 2], F32)
                    nc.vector.tensor_add(arg[:], mean[:], std[:])
                    attn = ypool.tile([128, 2], F32)
                    nc.scalar.activation(attn[:], arg[:], AF.Sigmoid)
                    for oh in range(2):
                        nc.vector.tensor_scalar_mul(y_sb[:, oh, :], y_sb[:, oh, :],
                                                    attn[:, oh:oh + 1])
                        nc.sync.dma_start(
                            _mk(y_d, oh * 128 * HW, [[HW, 128], [1, HW]]),
                            y_sb[:, oh, :])

    nc.compile()
    return nc


def _prep_shared(offset_w, deform_w):
    perm = [2 * i for i in range(9)] + [2 * i + 1 for i in range(9)]
    wp = np.asarray(offset_w, np.float32)[perm]
    wp2 = wp.reshape(18, 2, 128, 9)
    offw = np.ascontiguousarray(wp2.transpose(2, 1, 3, 0)).astype(ml_dtypes.bfloat16)

    wk = np.asarray(deform_w, np.float32).reshape(256, 256, 9)
    t = wk.reshape(256, 2, 128, 9).transpose(2, 3, 1, 0)
    w2 = np.ascontiguousarray(t.reshape(128, 18, 256)).astype(ml_dtypes.bfloat16)

    p = np.arange(128)
    blk = np.arange(32)
    kk = np.arange(9)
    i_pix = blk[None, :, None] * 2 + (p[:, None, None] // 64)
    j_pix = (p[:, None, None] % 64) + 0 * blk[None, :, None]
    basey = np.ascontiguousarray(np.broadcast_to(
        (i_pix + (kk // 3)[None, None, :] - 1), (128, 32, 9))).astype(np.float32)
    basex = np.ascontiguousarray(np.broadcast_to(
        (j_pix + (kk % 3)[None, None, :] - 1), (128, 32, 9))).astype(np.float32)

    idf = np.eye(128, dtype=np.float32)
    idb = np.eye(128, dtype=np.float32).astype(ml_dtypes.bfloat16)
    sel = np.zeros((36, 36, 128), np.float32)
    for r in range(36):
        sel[r, r, :] = 1.0
    sel = sel.astype(ml_dtypes.bfloat16)
    return dict(offw=offw, w2=w2, basey=basey, basex=basex, idf=idf, idb=idb,
                sel=sel)


_CACHE = {}


def kernel(x, offset_w, deform_w):
    x = np.asarray(x, np.float32)
    B = x.shape[0]
    assert x.shape == (8, 256, 64, 64)

    if "nc" not in _CACHE:
        _CACHE["nc"] = build_program()
    nc = _CACHE["nc"]

    shared = _prep_shared(offset_w, deform_w)
    in_maps = []
    for b in range(B):
        m = dict(shared)
        m["x"] = np.ascontiguousarray(x[b].reshape(256, HW))
        in_maps.append(m)

    from concourse.bass_utils import run_bass_kernel_spmd
    res = run_bass_kernel_spmd(nc, in_maps, core_ids=list(range(N_CORES)))
    out = np.stack([np.asarray(res.results[b]["y"], dtype=np.float32)
                    .reshape(256, 64, 64) for b in range(B)])
    return out


if __name__ == "__main__":
    d = np.load("/root/problem/ref_cache.npz")
    out = kernel(d["x"], d["offset_w"], d["deform_w"])
    err = np.abs(out - d["expected"]).max() / np.abs(d["expected"]).max()
    print("rel err vs cached ref:", err)


# revision 28
# speedup vs baseline: 1.2214x; 1.2214x over previous
"""DeformMCALayer Trainium2 kernel: 8-way data-parallel over batch.

kernel(**inputs) takes the FULL inputs (x [8,256,64,64], offset_w [18,256,3,3],
deform_w [256,256,3,3]) and returns the FULL output [8,256,64,64] (float32).

Per core (one image):
  1. x cast to bf16 (SWDGE cast DMA) -> xbf; offset conv (3x3 bf16 matmuls,
     PSUM-accumulated), offsets PE-transposed per conv chunk
  2. DVE index/bilinear-weight math (floor via 1.5*2^23 magic, clip, validity
     masks). Out-of-bounds corners are weight-redistributed (ex/ey edge
     tricks) so ONE clipped flat index per (tap,pixel) suffices.
  3. xbf transposed to pixel-major via 32 xbar DMA-transposes, then
     duplicated 4x into xt4 DRAM rows: row r holds pixels [r,r+1,r+64,r+65]
     (the 4 bilinear corners) so one SWDGE gather descriptor fetches all
     4 corners (2KB).
  4. w4 bilinear weights DMA'd to DRAM pixel-major, reloaded as w4t2
     [18,2,4096] rows.
  5. main loop per (qt,k): TWO 512-idx TRANSPOSING dma_gathers (xbar spray)
     land the 4 corners channel-on-partition/pixel-on-free; PE one-hot
     sel-matmuls broadcast the per-pixel weight rows to 128 partitions
     (ACT copies PSUM->SBUF bf16); ONE DVE broadcast tensor_tensor mult
     (stride-0 cb dim) weights all 4 corners; 2 DVE adds fold corner pairs
     into patchA/patchB. No PE transposes, no per-corner tensor_scalars.
  6. deform matmul: 36-step PSUM accumulation (k x cb x A/B) vs w2;
     channel attention (mean + unbiased std -> sigmoid) fused on ACT
     reading PSUM; final scale + store
"""
import sys
sys.path.insert(0, "/opt/trn_rl_repo")
import numpy as np
import ml_dtypes

import concourse.bacc as bacc
import concourse.mybir as mybir
from concourse.tile import TileContext
from concourse.ap import AP

F32 = mybir.dt.float32
BF16 = mybir.dt.bfloat16
I16 = mybir.dt.int16
OP = mybir.AluOpType
AF = mybir.ActivationFunctionType

H = W = 64
HW = 4096
K = 9
NQT = 4
QPIX = 1024
XT4ROWS = 4161   # physical rows 0..4160; gather reads rows 65..4160
N_CORES = 8
MAGIC = float(3 * 2 ** 22)  # 1.5*2^23 round-to-int magic (|x| < 2^22)


def _mk(ap_or_handle, extra_offset, dims):
    if isinstance(ap_or_handle, AP):
        t, off = ap_or_handle.tensor, ap_or_handle.offset
    else:
        a = ap_or_handle.ap()
        t, off = a.tensor, a.offset
    return AP(t, off + extra_offset, [list(d) for d in dims])


def build_program(repeat=1):
    nc = bacc.Bacc("TRN2", target_bir_lowering=False, debug=False, num_devices=1,
                   num_swdge_queues=4)

    x_d = nc.dram_tensor("x", [256, HW], F32, kind="ExternalInput")
    offw_d = nc.dram_tensor("offw", [128, 2, K, 18], BF16, kind="ExternalInput")
    w2_d = nc.dram_tensor("w2", [128, 18, 256], BF16, kind="ExternalInput")
    basey_d = nc.dram_tensor("basey", [128, 32, K], F32, kind="ExternalInput")
    basex_d = nc.dram_tensor("basex", [128, 32, K], F32, kind="ExternalInput")
    idf_d = nc.dram_tensor("idf", [128, 128], F32, kind="ExternalInput")
    idb_d = nc.dram_tensor("idb", [128, 128], BF16, kind="ExternalInput")
    sel_d = nc.dram_tensor("sel", [36, 36, 128], BF16, kind="ExternalInput")

    xbf_d = nc.dram_tensor("xbf", [256, HW], BF16, kind="Internal")
    xt4_d = nc.dram_tensor("xt4", [XT4ROWS, 1024], BF16, kind="Internal")
    y_d = nc.dram_tensor("y", [256, HW], BF16, kind="ExternalOutput")

    with TileContext(nc) as tc:
        for _rep in range(repeat):
            with tc.tile_pool(name="const", bufs=1) as cpool:
                w2_sb = cpool.tile([128, 18, 256], BF16)
                nc.sync.dma_start(w2_sb[:], w2_d[:])
                sel_sb = cpool.tile([36, 36, 128], BF16)
                nc.sync.dma_start(sel_sb[:], sel_d[:])
                idxw = cpool.tile([128, K, NQT, 64], I16)
                # w4T rows: w4T_sb[k*4+s, blk, p] = bilinear weight of pixel
                # blk*128+p for (tap k, corner s); (blk,p) = contiguous pix
                w4T_sb = cpool.tile([36, 32, 128], BF16)

                # idx pool opened BEFORE boot pool: its tiles sit below the
                # boot tiles in SBUF, so the index math never waits on an
                # anti-dependency against released boot buffers.
                with tc.tile_pool(name="idx", bufs=1) as ipool, \
                     tc.tile_pool(name="boot", bufs=1) as bpool, \
                     tc.tile_pool(name="psconv", bufs=2, space="PSUM") as psconv, \
                     tc.tile_pool(name="pstp", bufs=2, space="PSUM") as pstp:
                    idb_sb = ipool.tile([128, 128], BF16, name="idb")
                    nc.sync.dma_start(idb_sb[:], idb_d[:])
                    offw_sb = ipool.tile([128, 2, K, 18], BF16, name="offw")
                    nc.sync.dma_start(offw_sb[:], offw_d[:])
                    idf_sb = ipool.tile([128, 128], F32, name="idf")
                    nc.sync.dma_start(idf_sb[:], idf_d[:])
                    idxw16 = ipool.tile([16, K, NQT, 64], I16, name="idxw16")

                    # zero the xt4 tail corners no data write covers (0-weight
                    # corners must read 0.0, never NaN garbage): col1: row
                    # 4160; col2: rows 4097..4160; col3: rows 4096..4160
                    zpad = ipool.tile([65, 256], BF16, name="zpad")
                    nc.vector.memset(zpad[:], 0.0)
                    nc.sync.dma_start(
                        _mk(xt4_d, 4160 * 1024 + 256, [[1024, 1], [1, 256]]),
                        zpad[0:1, :])
                    nc.sync.dma_start(
                        _mk(xt4_d, 4097 * 1024 + 512, [[1024, 64], [1, 256]]),
                        zpad[0:64, :])
                    nc.sync.dma_start(
                        _mk(xt4_d, 4096 * 1024 + 768, [[1024, 65], [1, 256]]),
                        zpad[:])

                    basey_sb = ipool.tile([128, 32, K], F32, name="basey")
                    nc.sync.dma_start(basey_sb[:], basey_d[:])
                    basex_sb = ipool.tile([128, 32, K], F32, name="basex")
                    nc.sync.dma_start(basex_sb[:], basex_d[:])
                    offT = ipool.tile([128, 32, 18], F32, name="offT")
                    w4 = ipool.tile([128, 32, K, 4], BF16, name="w4")

                    # warm the PE HAM clock gate during the input DMAs so the
                    # conv runs at 2.4 GHz
                    for _w in range(24):
                        ps_w = pstp.tile([128, 128], BF16, tag="tpw4")
                        nc.tensor.transpose(ps_w[:], idb_sb[:], idb_sb[:])

                    # conv input (padded borders): load x f32 on the sync
                    # HWDGE queue, cast to bf16 on DVE (keeps SWDGE free so
                    # the 72 gathers own all 8 DMASW sem slots with a fixed
                    # queue-per-slot map)
                    x_pad = bpool.tile([128, 2, 66, 66], BF16)
                    nc.vector.memset(x_pad[:, :, 0, :], 0.0)
                    nc.vector.memset(x_pad[:, :, 65, :], 0.0)
                    nc.vector.memset(x_pad[:, :, 1:65, 0:1], 0.0)
                    nc.vector.memset(x_pad[:, :, 1:65, 65:66], 0.0)
                    x_sb = bpool.tile([128, 2, HW], F32)
                    for cb in range(2):
                        nc.sync.dma_start(
                            x_sb[:, cb, :],
                            _mk(x_d, cb * 128 * HW, [[HW, 128], [1, HW]]))
                    xbf_sb = bpool.tile([128, 2, HW], BF16)
                    nc.vector.tensor_copy(xbf_sb[:], x_sb[:])
                    xpi = _mk(x_pad[:], 67,
                              [list(x_pad[:].ap[0]), [4356, 2], [66, 64],
                               [1, 64]])
                    xbi = _mk(xbf_sb[:], 0,
                              [list(xbf_sb[:].ap[0]), [HW, 2], [64, 64],
                               [1, 64]])
                    nc.vector.tensor_copy(xpi, xbi)
                    nc.scalar.dma_start(
                        _mk(xbf_d, 0, [[HW, 128], [128 * HW, 2], [1, HW]]),
                        xbf_sb[:])

                    # pixel-major bf16 x via 32 xbar DMA-transposes (sync
                    # HWDGE queue, overlapping the conv on PE)
                    xT_sb = bpool.tile([128, 32, 256], BF16)
                    for b in range(32):
                        src = _mk(xbf_d, b * 128, [[HW, 256], [1, 128]])
                        nc.sync.dma_start_transpose(xT_sb[:, b, :], src)
                    # duplicate 4x into xt4: pixel j=128b+p lands at
                    # col c, physical row j + {65,64,1,0}[c]; writes split
                    # across the gpsimd + scalar DMA queues to overlap
                    for c, pr0 in enumerate([65, 64, 1, 0]):
                        dst = _mk(xt4_d, pr0 * 1024 + c * 256,
                                  [[1024, 128], [128 * 1024, 32], [1, 256]])
                        eng = nc.sync if c < 2 else nc.scalar
                        eng.dma_start(dst, xT_sb[:])

                    off_sb = bpool.tile([18, HW], F32)
                    for chk in range(8):
                        ps_conv = psconv.tile([18, 512], F32, tag="conv")
                        r0 = chk * 8
                        idx = 0
                        for cb in range(2):
                            for k in range(K):
                                ky, kx = k // 3, k % 3
                                rhs = x_pad[:, cb, r0 + ky: r0 + ky + 8, kx: kx + 64]
                                nc.tensor.matmul(
                                    ps_conv[:], offw_sb[:, cb, k, :], rhs,
                                    start=(idx == 0), stop=(idx == 17))
                                idx += 1
                        nc.scalar.copy(off_sb[:, chk * 512:(chk + 1) * 512], ps_conv[:])
                        for bb in range(4):
                            b = chk * 4 + bb
                            ps_t = pstp.tile([128, 18], F32, tag="tp18")
                            nc.tensor.transpose(
                                ps_t[:], off_sb[:, b * 128:(b + 1) * 128],
                                idf_sb[0:18, 0:18])
                            nc.scalar.copy(offT[:, b, :], ps_t[:])

                    # ---------------- index & weight math ----------------
                    def it(name):
                        return ipool.tile([128, 32, K], F32, tag=name, name=name)

                    sy = it("sy"); sx = it("sx")
                    nc.vector.tensor_add(sy[:], basey_sb[:], offT[:, :, 0:K])
                    nc.vector.tensor_add(sx[:], basex_sb[:], offT[:, :, K:18])

                    def floor_(s_t, name):
                        t = it(name + "_t"); c = it(name + "_c")
                        f = it(name + "_f"); l = it(name + "_l")
                        nc.vector.tensor_scalar_add(t[:], s_t[:], MAGIC)
                        nc.vector.tensor_scalar_sub(t[:], t[:], MAGIC)
                        nc.vector.tensor_tensor(c[:], t[:], s_t[:], OP.is_gt)
                        nc.vector.tensor_sub(f[:], t[:], c[:])
                        nc.vector.tensor_sub(l[:], s_t[:], f[:])
                        return f, l

                    y0, ly = floor_(sy, "y")
                    x0, lx = floor_(sx, "x")

                    yc0 = it("yc0"); xc0 = it("xc0")
                    nc.vector.tensor_scalar(yc0[:], y0[:], 0.0, 63.0, OP.max, OP.min)
                    nc.vector.tensor_scalar(xc0[:], x0[:], 0.0, 63.0, OP.max, OP.min)

                    # masks: v = in [0,63], c62 = in [0,62], e = (== -1)
                    def masks(src_t, name):
                        a = it(name + "_a"); b = it(name + "_b"); v = it(name + "_v")
                        c2 = it(name + "_c2"); c62 = it(name + "_c62")
                        e = it(name + "_e")
                        nc.vector.tensor_scalar(a[:], src_t[:], 0.0, None, OP.is_ge)
                        nc.vector.tensor_scalar(b[:], src_t[:], 63.0, None, OP.is_le)
                        nc.vector.tensor_mul(v[:], a[:], b[:])
                        nc.vector.tensor_scalar(c2[:], src_t[:], 62.0, None, OP.is_le)
                        nc.vector.tensor_mul(c62[:], a[:], c2[:])
                        nc.vector.tensor_scalar(e[:], src_t[:], -1.0, None, OP.is_equal)
                        return v, c62, e

                    vy0, cy62, ey = masks(y0, "my")
                    vx0, cx62, ex = masks(x0, "mx")

                    oly = it("oly"); olx = it("olx")
                    nc.vector.tensor_scalar(oly[:], ly[:], -1.0, 1.0, OP.mult, OP.add)
                    nc.vector.tensor_scalar(olx[:], lx[:], -1.0, 1.0, OP.mult, OP.add)

                    # wy0 = (1-ly)*vy0 + ly*ey ; wy1 = ly*cy62 (same for x)
                    wy0 = it("wy0"); wy1 = it("wy1"); wx0 = it("wx0"); wx1 = it("wx1")
                    t1 = it("t1"); t2 = it("t2")
                    nc.vector.tensor_mul(t1[:], oly[:], vy0[:])
                    nc.vector.tensor_mul(t2[:], ly[:], ey[:])
                    nc.vector.tensor_add(wy0[:], t1[:], t2[:])
                    nc.vector.tensor_mul(wy1[:], ly[:], cy62[:])
                    nc.vector.tensor_mul(t1[:], olx[:], vx0[:])
                    nc.vector.tensor_mul(t2[:], lx[:], ex[:])
                    nc.vector.tensor_add(wx0[:], t1[:], t2[:])
                    nc.vector.tensor_mul(wx1[:], lx[:], cx62[:])

                    for s, (a_t, b_t) in enumerate([(wy0, wx0), (wy0, wx1),
                                                    (wy1, wx0), (wy1, wx1)]):
                        nc.vector.tensor_tensor(w4[:, :, :, s], a_t[:], b_t[:], OP.mult)

                    # w4 [128pix, blk, (k,s)] -> w4T [36(k,s), blk, 128]
                    # via 32 PE transposes (pix-major weight rows on-chip)
                    for blk in range(32):
                        pswt = pstp.tile([128, 128], BF16, tag="tpw4")
                        nc.tensor.transpose(
                            pswt[0:36, :],
                            _mk(w4[:], blk * 36,
                                [list(w4[:].ap[0]), [1, 36]]),
                            idb_sb[:])
                        nc.scalar.copy(w4T_sb[:, blk, :], pswt[0:36, :])

                    # physical row index: yc0*64 + xc0 + 65
                    ida = it("ida"); m1 = it("m1")
                    nc.vector.tensor_scalar(m1[:], yc0[:], 64.0, 65.0, OP.mult, OP.add)
                    nc.vector.tensor_add(ida[:], m1[:], xc0[:])

                    # reorder [p, blk=qt*8+g, k] -> idxf [p, qt, k, g]
                    idxf = ipool.tile([128, NQT, K, 8], F32, tag="idxf")
                    src_ap = _mk(ida[:], 0, [list(ida[:].ap[0]), [8 * K, NQT],
                                             [1, K], [K, 8]])
                    dst_ap = _mk(idxf[:], 0, [list(idxf[:].ap[0]), [72, NQT],
                                              [8, K], [1, 8]])
                    nc.vector.tensor_copy(dst_ap, src_ap)

                    # two-stage 16-wide transpose into SWDGE index layout,
                    # one 72-wide chunk per qt
                    T1_sb = ipool.tile([128, NQT, 128], F32, tag="T1")
                    for ch in range(NQT):
                        ps = pstp.tile([128, 128], F32, tag="tpw")
                        in_ap = _mk(idxf[:], ch * 72, [list(idxf[:].ap[0]), [1, 72]])
                        nc.tensor.transpose(ps[0:72, :], in_ap, idf_sb[:])
                        nc.scalar.copy(T1_sb[0:72, ch, :], ps[0:72, :])
                    for ch in range(NQT):
                        for q in range(8):
                            ps2f = pstp.tile([128, 128], F32, tag="tpw")
                            in2 = T1_sb[:, ch, q * 16: q * 16 + 16]
                            nc.tensor.transpose(ps2f[0:16, :], in2, idf_sb[:])
                            base = idxw16[:].offset + ch * 64 + q
                            pa = list(idxw16[:].ap[0])
                            pa[1] = 16
                            dims = [pa, [256, K], [8, 8]]
                            dst_ap = AP(idxw16[:].tensor, base, dims)
                            nc.vector.tensor_copy(dst_ap, ps2f[0:16, 0:72])
                    for cgrp in range(8):
                        nc.sync.dma_start(idxw[cgrp * 16:(cgrp + 1) * 16], idxw16[:])

                # ---------------- main: gather, weight, matmul ----------
                with tc.tile_pool(name="main", bufs=2) as mpool, \
                     tc.tile_pool(name="ybuf", bufs=1) as ypool, \
                     tc.tile_pool(name="pswb", bufs=4, space="PSUM") as pswb, \
                     tc.tile_pool(name="psmm", bufs=4, space="PSUM") as psmm:
                    y_sb = ypool.tile([128, 2, HW], BF16)
                    s1p = ypool.tile([128, 2, 8], F32, name="s1p")
                    s2p = ypool.tile([128, 2, 8], F32, name="s2p")
                    gsrc_ap = _mk(xt4_d, 0, [[1024, XT4ROWS], [1, 1024]])

                    it_no = 0
                    for qt in range(NQT):
                        # patchA/B [128, 9k, 2h, 2cb, 512] bf16, single buf
                        patchA = mpool.tile([128, K, 2, 2, 512], BF16,
                                            tag="patchA", bufs=1)
                        patchB = mpool.tile([128, K, 2, 2, 512], BF16,
                                            tag="patchB", bufs=1)

                        for k in range(K):
                            # TWO 512-idx transposing gathers: gxT layout
                            # [128c, 2h, 8e, 512i], e = s*2+cb
                            # per-queue tags: a tag's SWDGE completion
                            # semaphore (DMASW<q>) is locked to one queue, so
                            # each queue gets its own gxT tag (4-deep rotate)
                            # transposed gathers are only reliable on ONE
                            # SWDGE queue (multi-queue runs were flaky on HW:
                            # DMASW sem slots are shared across queues)
                            qn = 0
                            gxT = mpool.tile([128, 2, 8, 512], BF16,
                                             tag="gxT", bufs=4)
                            for h in range(2):
                                nc.gpsimd.dma_gather(
                                    gxT[:, h, :, :], gsrc_ap,
                                    idxw[:, k, qt, 32 * h:32 * h + 32],
                                    512, 512, 1024, elem_step=1024,
                                    transpose=True, queue_num=qn)
                            it_no += 1

                            # weight rows for (qt, k): PE sel-matmul
                            # broadcast w4T[k*4+s] row (contract over all
                            # 36 partitions, one-hot lhsT) -> 8 chunks of
                            # [128, 512]; ACT copies PSUM -> wbc bf16
                            wbc = mpool.tile([128, 4, 1024], BF16,
                                             tag="wbc", bufs=2)
                            for s4 in range(4):
                                for jc in range(2):
                                    psb = pswb.tile([128, 512], F32,
                                                    tag="wb")
                                    rhs = _mk(w4T_sb[:],
                                              (qt * 8 + jc * 4) * 128,
                                              [list(w4T_sb[:].ap[0]),
                                               [1, 512]])
                                    nc.tensor.matmul(
                                        psb[:],
                                        sel_sb[:, k * 4 + s4, :], rhs,
                                        start=True, stop=True)
                                    nc.scalar.copy(
                                        wbc[:, s4,
                                            jc * 512:(jc + 1) * 512],
                                        psb[:])

                            # broadcast mult: gxT *= wbc (in place), one op
                            # per gather half; in1 cb dim stride 0 (ISA caps
                            # APs at 3 free dims)
                            for h in range(2):
                                in0 = _mk(gxT[:], h * 4096,
                                          [list(gxT[:].ap[0]), [1, 4096]])
                                in1 = _mk(wbc[:], h * 512,
                                          [list(wbc[:].ap[0]),
                                           [1024, 4], [0, 2], [1, 512]])
                                nc.vector.tensor_tensor(in0, in0, in1, OP.mult)

                            # corner-pair adds -> patchA/B[:, k, h, cb, i]
                            pa = _mk(patchA[:], k * 2048,
                                     [list(patchA[:].ap[0]), [1, 2048]])
                            pb = _mk(patchB[:], k * 2048,
                                     [list(patchB[:].ap[0]), [1, 2048]])
                            s0 = _mk(gxT[:], 0 * 1024,
                                     [list(gxT[:].ap[0]), [4096, 2], [1, 1024]])
                            s1 = _mk(gxT[:], 1 * 1024,
                                     [list(gxT[:].ap[0]), [4096, 2], [1, 1024]])
                            s2 = _mk(gxT[:], 2 * 1024,
                                     [list(gxT[:].ap[0]), [4096, 2], [1, 1024]])
                            s3 = _mk(gxT[:], 3 * 1024,
                                     [list(gxT[:].ap[0]), [4096, 2], [1, 1024]])
                            nc.vector.tensor_tensor(pa, s0, s1, OP.add)
                            nc.vector.tensor_tensor(pb, s2, s3, OP.add)

                        # deform matmul: 36 steps (k x cb x A/B) per
                        # (h-chunk, oh); psum [128, 512]
                        for h in range(2):
                            for oh in range(2):
                                psd = psmm.tile([128, 512], F32, tag="mm")
                                idx = 0
                                for k in range(K):
                                    for cb in range(2):
                                        lhs = w2_sb[:, k * 2 + cb,
                                                    oh * 128:(oh + 1) * 128]
                                        for pt in (patchA, patchB):
                                            rhs = _mk(pt[:],
                                                      k * 2048 + h * 1024
                                                      + cb * 512,
                                                      [list(pt[:].ap[0]),
                                                       [1, 512]])
                                            nc.tensor.matmul(
                                                psd[:], lhs, rhs,
                                                start=(idx == 0),
                                                stop=(idx == 35))
                                            idx += 1
                                cidx = qt * 2 + h
                                nc.scalar.activation(
                                    y_sb[:, oh, qt * 1024 + h * 512:
                                         qt * 1024 + (h + 1) * 512],
                                    psd[:], AF.Copy,
                                    accum_out=s1p[:, oh, cidx:cidx + 1])
                                sqscr = mpool.tile([128, 512], BF16, tag="sq")
                                nc.scalar.activation(
                                    sqscr[:], psd[:], AF.Square,
                                    accum_out=s2p[:, oh, cidx:cidx + 1])

                    # ---------------- stats + scale ----------------
                    s1 = ypool.tile([128, 2], F32)
                    s2 = ypool.tile([128, 2], F32)
                    nc.vector.reduce_sum(s1[:], s1p[:], axis=mybir.AxisListType.X)
                    nc.vector.reduce_sum(s2[:], s2p[:], axis=mybir.AxisListType.X)
                    mean = ypool.tile([128, 2], F32)
                    nc.vector.tensor_scalar_mul(mean[:], s1[:], 1.0 / HW)
                    ss = ypool.tile([128, 2], F32)
                    nc.vector.tensor_mul(ss[:], s1[:], s1[:])
                    va = ypool.tile([128, 2], F32)
                    vb = ypool.tile([128, 2], F32)
                    var = ypool.tile([128, 2], F32)
                    nc.vector.tensor_scalar_mul(va[:], s2[:], 1.0 / (HW - 1))
                    nc.vector.tensor_scalar_mul(vb[:], ss[:], 1.0 / (HW * (HW - 1.0)))
                    nc.vector.tensor_sub(var[:], va[:], vb[:])
                    nc.vector.tensor_scalar_max(var[:], var[:], 0.0)
                    std = ypool.tile([128, 2], F32)
                    nc.scalar.sqrt(std[:], var[:])
                    arg = ypool.tile([128, 2], F32)
                    nc.vector.tensor_add(arg[:], mean[:], std[:])
                    attn = ypool.tile([128, 2], F32)
                    nc.scalar.activation(attn[:], arg[:], AF.Sigmoid)
                    for oh in range(2):
                        nc.vector.tensor_scalar_mul(y_sb[:, oh, :], y_sb[:, oh, :],
                                                    attn[:, oh:oh + 1])
                        nc.sync.dma_start(
                            _mk(y_d, oh * 128 * HW, [[HW, 128], [1, HW]]),
                            y_sb[:, oh, :])

    nc.compile()
    return nc


def _prep_shared(offset_w, deform_w):
    perm = [2 * i for i in range(9)] + [2 * i + 1 for i in range(9)]
    wp = np.asarray(offset_w, np.float32)[perm]
    wp2 = wp.reshape(18, 2, 128, 9)
    offw = np.ascontiguousarray(wp2.transpose(2, 1, 3, 0)).astype(ml_dtypes.bfloat16)

    wk = np.asarray(deform_w, np.float32).reshape(256, 256, 9)
    t = wk.reshape(256, 2, 128, 9).transpose(2, 3, 1, 0)
    w2 = np.ascontiguousarray(t.reshape(128, 18, 256)).astype(ml_dtypes.bfloat16)

    p = np.arange(128)
    blk = np.arange(32)
    kk = np.arange(9)
    i_pix = blk[None, :, None] * 2 + (p[:, None, None] // 64)
    j_pix = (p[:, None, None] % 64) + 0 * blk[None, :, None]
    basey = np.ascontiguousarray(np.broadcast_to(
        (i_pix + (kk // 3)[None, None, :] - 1), (128, 32, 9))).astype(np.float32)
    basex = np.ascontiguousarray(np.broadcast_to(
        (j_pix + (kk % 3)[None, None, :] - 1), (128, 32, 9))).astype(np.float32)

    idf = np.eye(128, dtype=np.float32)
    idb = np.eye(128, dtype=np.float32).astype(ml_dtypes.bfloat16)
    sel = np.zeros((36, 36, 128), np.float32)
    for r in range(36):
        sel[r, r, :] = 1.0
    sel = sel.astype(ml_dtypes.bfloat16)
    return dict(offw=offw, w2=w2, basey=basey, basex=basex, idf=idf, idb=idb,
                sel=sel)


_CACHE = {}


def kernel(x, offset_w, deform_w):
    x = np.asarray(x, np.float32)
    B = x.shape[0]
    assert x.shape == (8, 256, 64, 64)

    if "nc" not in _CACHE:
        _CACHE["nc"] = build_program()
    nc = _CACHE["nc"]

    shared = _prep_shared(offset_w, deform_w)
    in_maps = []
    for b in range(B):
        m = dict(shared)
        m["x"] = np.ascontiguousarray(x[b].reshape(256, HW))
        in_maps.append(m)

    from concourse.bass_utils import run_bass_kernel_spmd
    res = run_bass_kernel_spmd(nc, in_maps, core_ids=list(range(N_CORES)))
    out = np.stack([np.asarray(res.results[b]["y"], dtype=np.float32)
                    .reshape(256, 64, 64) for b in range(B)])
    return out


if __name__ == "__main__":
    d = np.load("/root/problem/ref_cache.npz")
    out = kernel(d["x"], d["offset_w"], d["deform_w"])
    err = np.abs(out - d["expected"]).max() / np.abs(d["expected"]).max()
    print("rel err vs cached ref:", err)
